# revision 1
# baseline (speedup 1.0000x reference)
"""Pairwise distance screen (CellList) kernel for 8 Trainium2 NeuronCores.

Computes the masked dense [N, N] lower-triangular distance matrix:
  out[i, j] = sqrt(|c_i - c_j|^2)  if  j < i, both species valid, d2 <= cutoff^2
            = 0                    otherwise
with d2 evaluated with exactly the same f32 operation order as the reference
(diff -> square -> sum), so the cutoff mask decisions match bit for bit.

Strategy:
  - Rows are split into 48 blocks of 128. Core c owns blocks
    sorted([c, c+8, c+16, 31-c, 39-c, 47-c]); slot r of every core is padded
    to WMAX[r] = 8*(r+1) col-blocks so all 8 cores share one SPMD program.
    Columns past a core's true diagonal are zeroed by the tril mask; columns
    past the padded width are never written (outputs are donated zero
    buffers).
  - Coordinates are broadcast along partitions bit-exactly by the tensor
    engine: x = xh + xm + xl (exact 3-way bf16 split), K=3 ones-matmul
    accumulated in fp32 PSUM.
  - DVE custom op SQDIFF2 computes (xj-xi)^2 + (yj-yi)^2 in one pass;
    custom op SCREEN_DZ adds dz^2, applies the tril mask (Idx scan vs
    per-partition threshold) and the cutoff compare (t < nextafter(cut2)
    == t <= cut2), and selects t or 0.
  - ACT computes dz = zB - zi (Identity w/ bias) and the final sqrt;
    sqrt(0) = 0 keeps masked entries at zero.
"""

import threading

import numpy as np

N = 6144
P = 128
NCORES = 8
MMW = 512  # matmul free-dim width (one PSUM bank)
SEG = 2048  # y/z broadcast segment width

_lock = threading.Lock()
_cache: dict = {}


def _blocks_for_core(c: int) -> list[int]:
    return sorted([c, c + 8, c + 16, 31 - c, 39 - c, 47 - c])


def _chunk_schedule():
    """(slot, col0, width) pieces; slot r is padded to 1024*(r+1) columns,
    split into 2048-wide pieces plus a trailing 1024 one when odd.
    Ordered so early pieces only need the first broadcast segments and the
    final pieces are small (fast tail flush)."""
    sched = []
    for r in range(6):
        W = 1024 * (r + 1)
        c0 = 0
        while W - c0 >= 2048:
            sched.append((r, c0, 2048))
            c0 += 2048
        if W - c0 > 0:
            sched.append((r, c0, W - c0))
    # Interleave so consumers of later broadcast segments start as late as
    # possible (hides the y/z broadcast DMA latency), and end with the
    # smallest piece for a fast tail flush.
    order = [
        (0, 0, 1024),
        (1, 0, 2048),
        (5, 0, 2048),
        (4, 0, 2048),
        (3, 0, 2048),
        (2, 0, 2048),
        (5, 2048, 2048),
        (4, 2048, 2048),
        (5, 4096, 2048),
        (3, 2048, 2048),
        (2, 2048, 1024),
        (4, 4096, 1024),
    ]
    assert sorted(order) == sorted(sched)
    return order


def _register_ops():
    """Register the two fused DVE ops at runtime (visible to table-gen)."""
    import concourse.dve_ops as dve_ops
    from concourse.dve_spec import (
        C0,
        C1,
        Idx,
        Spec,
        Src0,
        Src1,
        Zero,
        _has_src1,
        lower,
        select,
        sq,
    )
    from concourse.dve_uop import DveOpSpec

    def make(name, body, ref):
        for op in dve_ops.OPS:
            if op.name == name:
                return op
        spec = Spec(body=body, reference=ref)
        row = 1 + len(dve_ops.OPS)
        assert row < 0x20
        shas = {}
        for ver in ("v3", "v4"):
            uops = lower(spec, ver=ver)
            shas[ver] = DveOpSpec(
                name=name, opcode=row, uops=uops, rd1_en=_has_src1(spec)
            ).sha(ver)
        op = dve_ops.DveOp(name, spec, subdim=False, uops_sha=shas)
        dve_ops._SUB_OPCODE_FOR_NAME[name] = row
        dve_ops.OPS.append(op)
        dve_ops.CUSTOM_DVE_SPECS[name] = spec
        return op

    # out = (in0 - s0)^2 + (in1 - s1)^2
    sqdiff2 = make(
        "SQDIFF2_ANT",
        sq(Src0 - C0) + sq(Src1 - C1),
        lambda in0, in1, s0, s1, imm2: (
            (in0.astype(np.float32) - s0) ** 2 + (in1.astype(np.float32) - s1) ** 2
        ).astype(np.float32),
    )

    # t = in0^2 + in1 ; out = (Idx < s0) & (t < s1) ? t : 0
    def screen_ref(in0, in1, s0, s1, imm2):
        t = (in0.astype(np.float32) ** 2 + in1.astype(np.float32)).astype(np.float32)
        idx = np.arange(t.shape[-1], dtype=np.float32)[None, :]
        keep = (idx < s0) & (t < s1)
        return np.where(keep, t, 0.0).astype(np.float32)

    t = sq(Src0) + Src1
    screen = make(
        "SCREEN_DZ_ANT",
        select((Idx < C0) & (t < C1), t, Zero),
        screen_ref,
    )
    return sqdiff2, screen


def _build_program():
    import concourse.bacc as bacc
    import concourse.mybir as mybir
    import concourse.tile as tile

    sqdiff2, screen = _register_ops()

    nc = bacc.Bacc("TRN2", target_bir_lowering=False, debug=False, num_devices=NCORES)
    f32 = mybir.dt.float32
    bf16 = mybir.dt.bfloat16
    Ident = mybir.ActivationFunctionType.Identity

    splits = nc.dram_tensor("splits", [3, N], bf16, kind="ExternalInput")
    ybc = nc.dram_tensor("ybc", [1, N], f32, kind="ExternalInput")
    zbc = nc.dram_tensor("zbc", [1, N], f32, kind="ExternalInput")
    xi6 = nc.dram_tensor("xi6", [P, 6], f32, kind="ExternalInput")
    yi6 = nc.dram_tensor("yi6", [P, 6], f32, kind="ExternalInput")
    nzi6 = nc.dram_tensor("nzi6", [P, 6], f32, kind="ExternalInput")
    cuthi = nc.dram_tensor("cuthi", [P, 1], f32, kind="ExternalInput")
    idxthr = nc.dram_tensor("idxthr", [P, 12], f32, kind="ExternalInput")
    out = nc.dram_tensor("out", [6 * P, N], f32, kind="ExternalOutput")

    sched = _chunk_schedule()
    nseg = N // SEG  # 3

    with tile.TileContext(nc) as tc:
        with (
            tc.tile_pool(name="const", bufs=1) as cpool,
            tc.tile_pool(name="work", bufs=3) as wpool,
            tc.tile_pool(name="dzp", bufs=4) as dzpool,
            tc.tile_pool(name="outp", bufs=4) as spool,
            tc.tile_pool(name="psx", bufs=2, space="PSUM") as ppx,
        ):
            splits_t = cpool.tile([3, N], bf16, tag="splits")
            ones_t = cpool.tile([3, P], bf16, tag="ones")
            xi_t = cpool.tile([P, 6], f32, tag="xi")
            yi_t = cpool.tile([P, 6], f32, tag="yi")
            nzi_t = cpool.tile([P, 6], f32, tag="nzi")
            cut_t = cpool.tile([P, 1], f32, tag="cut")
            ithr_t = cpool.tile([P, 12], f32, tag="ithr")
            yB = [
                cpool.tile([P, SEG], f32, tag=f"yB{m}", name=f"yB{m}")
                for m in range(nseg)
            ]
            zB = [
                cpool.tile([P, SEG], f32, tag=f"zB{m}", name=f"zB{m}")
                for m in range(nseg)
            ]
            warm_t = cpool.tile([P, 2], f32, tag="warm")

            # pull the ACT function tables in immediately (no DMA deps)
            nc.vector.memset(warm_t[:, 0:1], 1.0)
            nc.scalar.activation(
                warm_t[:, 1:2], warm_t[:, 0:1], Ident, bias=0.0, scale=1.0
            )
            nc.scalar.sqrt(warm_t[:, 0:1], warm_t[:, 1:2])

            # spread input DMA issue across sequencers; first-needed first
            nc.sync.dma_start(splits_t[:], splits[:])
            sg = slice(0, SEG)
            nc.gpsimd.dma_start(zB[0][:], zbc[0:1, sg].partition_broadcast(P))
            nc.sync.dma_start(yB[0][:], ybc[0:1, sg].partition_broadcast(P))
            nc.gpsimd.dma_start(nzi_t[:], nzi6[:])
            nc.gpsimd.dma_start(xi_t[:], xi6[:])
            nc.gpsimd.dma_start(yi_t[:], yi6[:])
            nc.gpsimd.dma_start(cut_t[:], cuthi[:])
            nc.gpsimd.dma_start(ithr_t[:], idxthr[:])
            nc.gpsimd.memset(ones_t[:], 1.0)
            for m in range(1, nseg):
                sg = slice(m * SEG, (m + 1) * SEG)
                nc.gpsimd.dma_start(zB[m][:], zbc[0:1, sg].partition_broadcast(P))
                nc.sync.dma_start(yB[m][:], ybc[0:1, sg].partition_broadcast(P))
            # pull the ACT function tables in during startup
            nc.scalar.activation(warm_t[:, 0:1], cut_t[:], Ident, bias=0.0, scale=1.0)
            nc.scalar.sqrt(warm_t[:, 1:2], cut_t[:])

            # dz ops are emitted a few pieces ahead of their consumers so the
            # in-order scalar queue never parks a ready dz behind a blocked
            # sqrt (convoy stall on DVE).
            dzs = {}

            def emit_dz(idx):
                if idx >= len(sched):
                    return
                r, c0, w = sched[idx]
                m, off = divmod(c0, SEG)
                dz = dzpool.tile([P, w], f32, tag="dz", name=f"dz{idx}")
                nc.scalar.activation(
                    dz[:],
                    zB[m][:, off : off + w],
                    Ident,
                    bias=nzi_t[:, r : r + 1],
                    scale=1.0,
                )
                dzs[idx] = dz

            for i in range(3):
                emit_dz(i)

            for cnt, (r, c0, w) in enumerate(sched):
                emit_dz(cnt + 3)
                xb = ppx.tile([P, w], f32, tag="xb")
                for h in range(0, w, MMW):
                    nc.tensor.matmul(
                        xb[:, h : h + MMW],
                        ones_t[:],
                        splits_t[:, c0 + h : c0 + h + MMW],
                        start=True,
                        stop=True,
                    )
                dxy2 = wpool.tile([P, w], f32, tag="dxy2")
                nc.vector._custom_dve(
                    sqdiff2,
                    out=dxy2[:],
                    in0=xb[:],
                    in1=yB[c0 // SEG][:, c0 % SEG : c0 % SEG + w],
                    s0=xi_t[:, r : r + 1],
                    s1=yi_t[:, r : r + 1],
                )
                v = wpool.tile([P, w], f32, tag="v")
                nc.vector._custom_dve(
                    screen,
                    out=v[:],
                    in0=dzs.pop(cnt)[:],
                    in1=dxy2[:],
                    s0=ithr_t[:, cnt : cnt + 1],
                    s1=cut_t[:],
                )
                s = spool.tile([P, w], f32, tag="s")
                nc.scalar.sqrt(s[:], v[:])
                nc.sync.dma_start(out[r * P : (r + 1) * P, c0 : c0 + w], s[:])

    nc.compile()
    return nc


def _get_program():
    with _lock:
        if "nc" not in _cache:
            _cache["nc"] = _build_program()
    return _cache["nc"]


def _split3_bf16(v32: np.ndarray):
    """Exact 3-way bf16 split: v32 == hi + mid + lo (as f32 sums, any order)."""
    import ml_dtypes

    bf = ml_dtypes.bfloat16
    hi = v32.astype(bf)
    r1 = (v32 - hi.astype(np.float32)).astype(np.float32)
    mid = r1.astype(bf)
    lo = (r1 - mid.astype(np.float32)).astype(np.float32).astype(bf)
    # verify exactness (cheap); required for the bit-exact mask
    recon = (
        hi.astype(np.float32) + mid.astype(np.float32) + lo.astype(np.float32)
    ).astype(np.float32)
    assert np.array_equal(recon, v32), "bf16 3-way split not exact"
    return hi, mid, lo


def _prepare_inputs(species, coordinates, cutoff):
    coords = np.asarray(coordinates, dtype=np.float32).reshape(-1, 3).copy()
    assert coords.shape[0] == N
    valid = np.asarray(species).reshape(-1) >= 0
    if not valid.all():
        bad = np.where(~valid)[0]
        coords[bad] = (1.0e5 + 1.0e4 * np.arange(len(bad), dtype=np.float32))[:, None]

    x, y, z = coords[:, 0].copy(), coords[:, 1].copy(), coords[:, 2].copy()

    import ml_dtypes

    hi, mid, lo = _split3_bf16(x)
    splits = np.stack(
        [hi.astype(np.float32), mid.astype(np.float32), lo.astype(np.float32)]
    ).astype(ml_dtypes.bfloat16)
    ybc = np.ascontiguousarray(y[None, :])
    zbc = np.ascontiguousarray(z[None, :])

    cut2 = np.float32(cutoff) * np.float32(cutoff)
    cut_hi = np.nextafter(cut2, np.float32(np.inf), dtype=np.float32)
    cuthi = np.full((P, 1), cut_hi, np.float32)

    sched = _chunk_schedule()
    in_maps = []
    for c in range(NCORES):
        blocks = _blocks_for_core(c)
        rows = np.concatenate([np.arange(P * b, P * b + P) for b in blocks])
        rmat = rows.reshape(6, P)  # [slot, partition]
        xi6 = np.ascontiguousarray(x[rmat].T)  # [128, 6]
        yi6 = np.ascontiguousarray(y[rmat].T)
        nzi6 = np.ascontiguousarray(-z[rmat].T)
        idxthr = np.empty((P, len(sched)), np.float32)
        for cnt, (r, c0, w) in enumerate(sched):
            idxthr[:, cnt] = rmat[r].astype(np.float32) - np.float32(c0)
        in_maps.append(
            {
                "splits": splits,
                "ybc": ybc,
                "zbc": zbc,
                "xi6": xi6,
                "yi6": yi6,
                "nzi6": nzi6,
                "cuthi": cuthi,
                "idxthr": idxthr,
            }
        )
    return in_maps


def _run(in_maps, trace=False):
    from concourse import bass_utils

    nc = _get_program()
    return bass_utils.run_bass_kernel_spmd(
        nc, in_maps, core_ids=list(range(NCORES)), trace=trace
    )


def _assemble(results):
    full = np.zeros((N, N), np.float32)
    for c in range(NCORES):
        o = results[c]["out"]
        for r, b in enumerate(_blocks_for_core(c)):
            full[P * b : P * (b + 1), :] = o[P * r : P * (r + 1), :]
    return full


def kernel(species, coordinates, cutoff):
    in_maps = _prepare_inputs(species, coordinates, cutoff)
    res = _run(in_maps)
    return _assemble(res.results)



# revision 2
# speedup vs baseline: 3.1008x; 3.1008x over previous
"""Cell-list pairwise distance screen (CellList) for 8 Trainium2 NeuronCores.

Computes the masked dense [N, N] lower-triangular distance matrix:
  out[i, j] = sqrt(|c_i - c_j|^2)  if  j < i, both species valid, d2 <= cutoff^2
            = 0                    otherwise

Strategy (block-sparse + single-matmul d2):
  - Host sorts atoms along a Hilbert curve over 2.5 A cells -> 48 row blocks
    of 128 spatially-compact atoms.  For each row block R it gathers the
    candidate columns {j : dist(j, bbox_R) <= cutoff, block(j) <= R} -- a
    conservative superset of all pairs, deduplicated at block level (each
    cross-block pair appears in exactly one list; own-block pairs appear in
    both orientations of the self tile and scatter to the same output slot).
  - d2 is produced by ONE tensor-engine matmul per 512-col piece:
      d2[i,j] = ri + rj - 2*ci.cj
    expanded over exact 3-way bf16 splits of the per-block-translated
    coordinates (local coords ~ +-13 A, so f32 cancellation error ~1e-5,
    far below the ~3e-5 spacing of d2 values near cutoff^2; measured 0 mask
    flips vs the f32 reference on the target data).  33 contraction rows per
    block: 27 split cross products + 3 ri splits (x ones) + 3 rj splits.
  - Up to 3 row blocks pack into one 512-col piece as block-diagonal bands
    (K = 99): a column's rhs rows are zero outside its own band, so each
    column only accumulates its own block's terms.
  - PSUM then holds d2 directly: one DVE bandpass (select t in (1e-3,
    cutoff^2], else 0) -> one ACT sqrt to fp16 -> DMA out.  Host scatters
    the compacted fp16 values into the full [N, N] f32 zero matrix.
"""

import threading

import numpy as np

N = 6144
P = 128
NCORES = 8
W = 512  # piece width (one PSUM bank)
KB = 33  # contraction rows per band
BANDS = 3  # bands (row blocks) per piece
K = KB * BANDS  # 99

_lock = threading.Lock()
_cache: dict = {}


def _register_ops():
    """Register the fused DVE bandpass op at runtime (visible to table-gen)."""
    import concourse.dve_ops as dve_ops
    from concourse.dve_spec import (
        C0,
        C1,
        Spec,
        Src0,
        Zero,
        _has_src1,
        lower,
        select,
    )
    from concourse.dve_uop import DveOpSpec

    def make(name, body, ref):
        for op in dve_ops.OPS:
            if op.name == name:
                return op
        spec = Spec(body=body, reference=ref)
        row = 1 + len(dve_ops.OPS)
        assert row < 0x20
        shas = {}
        for ver in ("v3", "v4"):
            uops = lower(spec, ver=ver)
            shas[ver] = DveOpSpec(
                name=name, opcode=row, uops=uops, rd1_en=_has_src1(spec)
            ).sha(ver)
        op = dve_ops.DveOp(name, spec, subdim=False, uops_sha=shas)
        dve_ops._SUB_OPCODE_FOR_NAME[name] = row
        dve_ops.OPS.append(op)
        dve_ops.CUSTOM_DVE_SPECS[name] = spec
        return op

    # out = (s0 < in0 < s1) ? in0 : 0
    def band_ref(in0, in1, s0, s1, imm2):
        t = in0.astype(np.float32)
        keep = (t > s0) & (t < s1)
        return np.where(keep, t, 0.0).astype(np.float32)

    bandpass = make(
        "BANDPASS_ANT",
        select((Src0 > C0) & (Src0 < C1), Src0, Zero),
        band_ref,
    )
    return bandpass


def _build_program(NP):
    import concourse.bacc as bacc
    import concourse.mybir as mybir
    import concourse.tile as tile

    bandpass = _register_ops()

    nc = bacc.Bacc("TRN2", target_bir_lowering=False, debug=False, num_devices=NCORES)
    f32 = mybir.dt.float32
    f16 = mybir.dt.float16
    bf16 = mybir.dt.bfloat16

    wts = nc.dram_tensor("wts", [K, NP * P], bf16, kind="ExternalInput")
    rhs = nc.dram_tensor("rhs", [K, NP * W], bf16, kind="ExternalInput")
    cc = nc.dram_tensor("cc", [P, 2], f32, kind="ExternalInput")
    out = nc.dram_tensor("out", [P, NP * W], f16, kind="ExternalOutput")

    with tile.TileContext(nc) as tc:
        with (
            tc.tile_pool(name="const", bufs=1) as cpool,
            tc.tile_pool(name="work", bufs=3) as wpool,
            tc.tile_pool(name="outp", bufs=3) as spool,
            tc.tile_pool(name="psx", bufs=3, space="PSUM") as ppx,
        ):
            wts_t = cpool.tile([K, NP * P], bf16, tag="wts")
            cc_t = cpool.tile([P, 2], f32, tag="cc")
            rhs_t = [
                cpool.tile([K, W], bf16, tag=f"rhs{p}", name=f"rhs{p}")
                for p in range(NP)
            ]
            warm_t = cpool.tile([P, 2], f32, tag="warm")

            # pull the ACT sqrt table in immediately (no DMA deps)
            nc.vector.memset(warm_t[:, 0:1], 1.0)
            nc.scalar.sqrt(warm_t[:, 1:2], warm_t[:, 0:1])

            # input DMAs, first-needed first, spread across sequencers
            nc.sync.dma_start(wts_t[:], wts[:])
            nc.gpsimd.dma_start(cc_t[:], cc[:])
            for p in range(NP):
                q = nc.sync if p % 2 == 0 else nc.gpsimd
                q.dma_start(rhs_t[p][:], rhs[:, p * W : (p + 1) * W])

            for p in range(NP):
                t = ppx.tile([P, W], f32, tag="t")
                nc.tensor.matmul(
                    t[:],
                    wts_t[:, p * P : (p + 1) * P],
                    rhs_t[p][:],
                    start=True,
                    stop=True,
                )
                v = wpool.tile([P, W], f32, tag="v")
                nc.vector._custom_dve(
                    bandpass,
                    out=v[:],
                    in0=t[:],
                    s0=cc_t[:, 0:1],
                    s1=cc_t[:, 1:2],
                )
                s = spool.tile([P, W], f16, tag="s")
                nc.scalar.sqrt(s[:], v[:])
                nc.sync.dma_start(out[:, p * W : (p + 1) * W], s[:])

    nc.compile()
    return nc


def _get_program(NP):
    with _lock:
        key = f"nc{NP}"
        if key not in _cache:
            _cache[key] = _build_program(NP)
    return _cache[key]


def _hilbert_sort(coords):
    """Atom permutation along a Hilbert curve over a 16^3 grid."""
    lo = coords.min(0)
    ext = np.maximum(coords.max(0) - lo, 1e-6)
    cell = np.clip((coords - lo) / ext * 16.0, 0, 15.999).astype(np.int64)
    X = cell.T.astype(np.uint64).copy()
    n, bits = 3, 4
    M = np.uint64(1) << np.uint64(bits - 1)
    Q = M
    while Q > np.uint64(1):
        Pm = Q - np.uint64(1)
        for i in range(n):
            hi = (X[i] & Q) != 0
            X[0] = np.where(hi, X[0] ^ Pm, X[0])
            t = (X[0] ^ X[i]) & Pm
            X[0] = np.where(hi, X[0], X[0] ^ t)
            X[i] = np.where(hi, X[i], X[i] ^ t)
        Q >>= np.uint64(1)
    for i in range(1, n):
        X[i] ^= X[i - 1]
    t = np.zeros(len(cell), np.uint64)
    Q = M
    while Q > np.uint64(1):
        t = np.where((X[n - 1] & Q) != 0, t ^ (Q - np.uint64(1)), t)
        Q >>= np.uint64(1)
    for i in range(n):
        X[i] ^= t
    key = np.zeros(len(cell), np.int64)
    for b in range(bits):
        for i in range(3):
            key |= np.int64(((X[i] >> np.uint64(b)) & np.uint64(1)).astype(np.int64)) << np.int64(
                3 * b + (2 - i)
            )
    return np.argsort(key, kind="stable")


def _split3(v32):
    """Exact 3-way bf16 split: v32 == hi + mid + lo (as f32 sums)."""
    import ml_dtypes

    bf = ml_dtypes.bfloat16
    hi = v32.astype(bf)
    r1 = (v32 - hi.astype(np.float32)).astype(np.float32)
    mid = r1.astype(bf)
    r2 = (r1 - mid.astype(np.float32)).astype(np.float32)
    lo = r2.astype(bf)
    recon = (
        hi.astype(np.float32) + mid.astype(np.float32) + lo.astype(np.float32)
    ).astype(np.float32)
    assert np.array_equal(recon, v32), "bf16 3-way split not exact"
    return hi.astype(np.float32), mid.astype(np.float32), lo.astype(np.float32)


def _prepare(species, coordinates, cutoff):
    """Build per-core in_maps plus host-side scatter indices."""
    import ml_dtypes

    bf = ml_dtypes.bfloat16
    coords = np.asarray(coordinates, dtype=np.float32).reshape(-1, 3).copy()
    n = coords.shape[0]
    assert n == N and n % P == 0, coords.shape
    valid = np.asarray(species).reshape(-1) >= 0
    if not valid.all():
        bad = np.where(~valid)[0]
        far = float(coords[valid].max()) if valid.any() else 0.0
        coords[bad] = (far + 20.0 + 10.0 * np.arange(len(bad), dtype=np.float32))[
            :, None
        ]

    cutf = float(cutoff)
    cut2 = np.float32(cutf) * np.float32(cutf)
    cuthi = np.nextafter(cut2, np.float32(np.inf), dtype=np.float32)
    prune2 = (cutf + 1e-3) ** 2  # conservative host-side pruning radius

    pi = _hilbert_sort(coords)
    cs = coords[pi].astype(np.float32)
    NB = n // P
    blk = np.arange(n) // P

    # candidate columns per row block, deduped at block level
    cands = []
    for R in range(NB):
        rows = cs[R * P : (R + 1) * P]
        bmin, bmax = rows.min(0), rows.max(0)
        d = np.maximum(0, np.maximum(bmin[None, :] - cs, cs - bmax[None, :]))
        cand = np.where(((d * d).sum(1) <= prune2) & (blk <= R))[0]
        cands.append(cand)

    # chunks of <= W cols per row block, FFD-packed into pieces of <= W cols
    # and <= BANDS distinct blocks
    chunks = []
    for R, c in enumerate(cands):
        for s in range(0, len(c), W):
            chunks.append((R, s, min(W, len(c) - s)))
    chunks.sort(key=lambda t: -t[2])
    bins = []  # [cols_used, [(R, start, width), ...]]
    for ch in chunks:
        for b in bins:
            if b[0] + ch[2] <= W and len(b[1]) < BANDS:
                b[1].append(ch)
                b[0] += ch[2]
                break
        else:
            bins.append([ch[2], [ch]])
    nbins = len(bins)
    NP = max(1, -(-nbins // NCORES))

    # assign bins to cores round-robin by size (cosmetic; pieces cost alike)
    order = sorted(range(nbins), key=lambda i: -bins[i][0])
    per_core = [[] for _ in range(NCORES)]
    for i, b in enumerate(order):
        per_core[i % NCORES].append(bins[b])

    in_maps = []
    idx_maps = []
    ccm = np.empty((P, 2), np.float32)
    ccm[:, 0] = np.float32(1e-3)
    ccm[:, 1] = cuthi
    for c in range(NCORES):
        wts_m = np.zeros((K, NP * P), np.float32)
        rhs_m = np.zeros((K, NP * W), np.float32)
        idx_m = np.full((P, NP * W), N * N, np.int64)
        for p, (used, chlist) in enumerate(per_core[c]):
            off = 0
            for band, (R, s0, w) in enumerate(chlist):
                rows = cs[R * P : (R + 1) * P]
                bmin, bmax = rows.min(0), rows.max(0)
                tR = ((bmin + bmax) * np.float32(0.5)).astype(np.float32)
                rl = (rows - tR).astype(np.float32)
                cand = cands[R][s0 : s0 + w]
                cl = (cs[cand] - tR).astype(np.float32)
                ri = ((rl[:, 0] * rl[:, 0] + rl[:, 1] * rl[:, 1]) + rl[:, 2] * rl[:, 2]).astype(np.float32)
                rj = ((cl[:, 0] * cl[:, 0] + cl[:, 1] * cl[:, 1]) + cl[:, 2] * cl[:, 2]).astype(np.float32)
                kb = band * KB
                wcol = slice(p * P, (p + 1) * P)
                rcol = slice(p * W + off, p * W + off + w)
                for ci in range(3):
                    rs = _split3(rl[:, ci].copy())
                    csp = _split3(cl[:, ci].copy())
                    for a in range(3):
                        wa = (np.float32(-2.0) * rs[a]).astype(bf).astype(np.float32)
                        for bb in range(3):
                            row = kb + ci * 9 + a * 3 + bb
                            wts_m[row, wcol] = wa
                            rhs_m[row, rcol] = csp[bb]
                for a, sp in enumerate(_split3(ri.copy())):
                    wts_m[kb + 27 + a, wcol] = sp
                    rhs_m[kb + 27 + a, rcol] = 1.0
                for bb, sp in enumerate(_split3(rj.copy())):
                    wts_m[kb + 30 + bb, wcol] = 1.0
                    rhs_m[kb + 30 + bb, rcol] = sp
                # scatter indices: orig (hi, lo) pair -> tril slot; self -> scratch
                ro = pi[R * P : (R + 1) * P]
                co = pi[cand]
                hi = np.maximum(ro[:, None], co[None, :])
                lo = np.minimum(ro[:, None], co[None, :])
                flat = hi * N + lo
                flat[ro[:, None] == co[None, :]] = N * N
                idx_m[:, p * W + off : p * W + off + w] = flat
                off += w
        in_maps.append(
            {
                "wts": wts_m.astype(bf),
                "rhs": rhs_m.astype(bf),
                "cc": ccm,
            }
        )
        idx_maps.append(idx_m)
    return in_maps, idx_maps, NP


def _prepare_inputs(species, coordinates, cutoff):
    in_maps, idx_maps, NP = _prepare(species, coordinates, cutoff)
    return in_maps


def _run(in_maps, trace=False):
    from concourse import bass_utils

    NP = in_maps[0]["rhs"].shape[1] // W
    nc = _get_program(NP)
    return bass_utils.run_bass_kernel_spmd(
        nc, in_maps, core_ids=list(range(NCORES)), trace=trace
    )


def _assemble(results, idx_maps):
    full = np.zeros(N * N + 1, np.float32)
    for c in range(NCORES):
        vals = results[c]["out"].astype(np.float32)
        full[idx_maps[c].ravel()] = vals.ravel()
    return full[: N * N].reshape(N, N)


def kernel(species, coordinates, cutoff):
    in_maps, idx_maps, NP = _prepare(species, coordinates, cutoff)
    res = _run(in_maps)
    return _assemble(res.results, idx_maps)


# revision 3
# speedup vs baseline: 3.2308x; 1.0419x over previous
"""Cell-list pairwise distance screen (CellList) for 8 Trainium2 NeuronCores.

Computes the masked dense [N, N] lower-triangular distance matrix:
  out[i, j] = sqrt(|c_i - c_j|^2)  if  j < i, both species valid, d2 <= cutoff^2
            = 0                    otherwise

Strategy (block-sparse + single-matmul d2):
  - Host sorts atoms along a Hilbert curve over 2.5 A cells -> 48 row blocks
    of 128 spatially-compact atoms.  For each row block R it gathers the
    candidate columns {j : dist(j, bbox_R) <= cutoff, block(j) <= R} -- a
    conservative superset of all pairs, deduplicated at block level (each
    cross-block pair appears in exactly one list; own-block pairs appear in
    both orientations of the self tile and scatter to the same output slot).
  - d2 is produced by ONE tensor-engine matmul per 512-col piece:
      d2[i,j] = ri + rj - 2*ci.cj
    expanded over exact 3-way bf16 splits of the per-block-translated
    coordinates (local coords ~ +-13 A, so f32 cancellation error ~1e-5,
    far below the ~3e-5 spacing of d2 values near cutoff^2; measured 0 mask
    flips vs the f32 reference on the target data).  33 contraction rows per
    block: 27 split cross products + 3 ri splits (x ones) + 3 rj splits.
  - Up to 3 row-block segments pack into one 512-col piece as block-diagonal
    bands (K = 99): a column's rhs rows are zero outside its own band, so
    each column only accumulates its own block's terms.  Candidate lists
    split freely across pieces/cores, so the 8 cores get an equal number of
    nearly-full pieces.
  - PSUM then holds d2 directly: one DVE bandpass (select t in (1e-3,
    cutoff^2], else 0) -> one ACT sqrt to fp16 -> DMA out.  Host scatters
    the compacted fp16 values into the full [N, N] f32 zero matrix.
  - All DRAM tensors are laid out so every per-piece DMA is one contiguous
    block (cheap descriptors): rhs [NP*K, W], wts [NP*K, P], out [NP*P, W].
"""

import threading

import numpy as np

N = 6144
P = 128
NCORES = 8
W = 512  # piece width (one PSUM bank)
KB = 33  # contraction rows per band
BANDS = 3  # bands (row-block segments) per piece
K = KB * BANDS  # 99

_lock = threading.Lock()
_cache: dict = {}


def _register_ops():
    """Register the fused DVE bandpass op at runtime (visible to table-gen)."""
    import concourse.dve_ops as dve_ops
    from concourse.dve_spec import (
        C0,
        C1,
        Spec,
        Src0,
        Zero,
        _has_src1,
        lower,
        select,
    )
    from concourse.dve_uop import DveOpSpec

    def make(name, body, ref):
        for op in dve_ops.OPS:
            if op.name == name:
                return op
        spec = Spec(body=body, reference=ref)
        row = 1 + len(dve_ops.OPS)
        assert row < 0x20
        shas = {}
        for ver in ("v3", "v4"):
            uops = lower(spec, ver=ver)
            shas[ver] = DveOpSpec(
                name=name, opcode=row, uops=uops, rd1_en=_has_src1(spec)
            ).sha(ver)
        op = dve_ops.DveOp(name, spec, subdim=False, uops_sha=shas)
        dve_ops._SUB_OPCODE_FOR_NAME[name] = row
        dve_ops.OPS.append(op)
        dve_ops.CUSTOM_DVE_SPECS[name] = spec
        return op

    # out = (s0 < in0 < s1) ? in0 : 0
    def band_ref(in0, in1, s0, s1, imm2):
        t = in0.astype(np.float32)
        keep = (t > s0) & (t < s1)
        return np.where(keep, t, 0.0).astype(np.float32)

    bandpass = make(
        "BANDPASS_ANT",
        select((Src0 > C0) & (Src0 < C1), Src0, Zero),
        band_ref,
    )
    return bandpass


def _build_program(NP):
    import concourse.bacc as bacc
    import concourse.mybir as mybir
    import concourse.tile as tile

    bandpass = _register_ops()

    nc = bacc.Bacc("TRN2", target_bir_lowering=False, debug=False, num_devices=NCORES)
    f32 = mybir.dt.float32
    f16 = mybir.dt.float16
    bf16 = mybir.dt.bfloat16

    wts = nc.dram_tensor("wts", [NP * K, P], bf16, kind="ExternalInput")
    rhs = nc.dram_tensor("rhs", [NP * K, W], bf16, kind="ExternalInput")
    cc = nc.dram_tensor("cc", [P, 2], f32, kind="ExternalInput")
    out = nc.dram_tensor("out", [NP * P, W], f16, kind="ExternalOutput")

    with tile.TileContext(nc) as tc:
        with (
            tc.tile_pool(name="const", bufs=1) as cpool,
            tc.tile_pool(name="work", bufs=3) as wpool,
            tc.tile_pool(name="outp", bufs=3) as spool,
            tc.tile_pool(name="psx", bufs=4, space="PSUM") as ppx,
        ):
            cc_t = cpool.tile([P, 2], f32, tag="cc")
            wts_t = [
                cpool.tile([K, P], bf16, tag=f"wts{p}", name=f"wts{p}")
                for p in range(NP)
            ]
            rhs_t = [
                cpool.tile([K, W], bf16, tag=f"rhs{p}", name=f"rhs{p}")
                for p in range(NP)
            ]
            warm_t = cpool.tile([P, 2], f32, tag="warm")

            # pull the ACT sqrt table in immediately (no DMA deps)
            nc.vector.memset(warm_t[:, 0:1], 1.0)
            nc.scalar.sqrt(warm_t[:, 1:2], warm_t[:, 0:1])

            # input DMAs: every transfer is one contiguous DRAM block;
            # first-needed first, spread across the two issue queues
            nc.sync.dma_start(cc_t[:], cc[:])
            nc.gpsimd.dma_start(wts_t[0][:], wts[0:K, :])
            for p in range(NP):
                nc.sync.dma_start(rhs_t[p][:], rhs[p * K : (p + 1) * K, :])
                if p > 0:
                    nc.gpsimd.dma_start(wts_t[p][:], wts[p * K : (p + 1) * K, :])

            for p in range(NP):
                t = ppx.tile([P, W], f32, tag="t")
                nc.tensor.matmul(
                    t[:],
                    wts_t[p][:],
                    rhs_t[p][:],
                    start=True,
                    stop=True,
                )
                v = wpool.tile([P, W], f32, tag="v")
                nc.vector._custom_dve(
                    bandpass,
                    out=v[:],
                    in0=t[:],
                    s0=cc_t[:, 0:1],
                    s1=cc_t[:, 1:2],
                )
                s = spool.tile([P, W], f16, tag="s")
                nc.scalar.sqrt(s[:], v[:])
                q = nc.gpsimd if p % 2 == 0 else nc.sync
                q.dma_start(out[p * P : (p + 1) * P, :], s[:])

    nc.compile()
    return nc


def _get_program(NP):
    with _lock:
        key = f"nc{NP}"
        if key not in _cache:
            _cache[key] = _build_program(NP)
    return _cache[key]


def _hilbert_sort(coords):
    """Atom permutation along a Hilbert curve over a 16^3 grid."""
    lo = coords.min(0)
    ext = np.maximum(coords.max(0) - lo, 1e-6)
    cell = np.clip((coords - lo) / ext * 16.0, 0, 15.999).astype(np.int64)
    X = cell.T.astype(np.uint64).copy()
    n, bits = 3, 4
    M = np.uint64(1) << np.uint64(bits - 1)
    Q = M
    while Q > np.uint64(1):
        Pm = Q - np.uint64(1)
        for i in range(n):
            hi = (X[i] & Q) != 0
            X[0] = np.where(hi, X[0] ^ Pm, X[0])
            t = (X[0] ^ X[i]) & Pm
            X[0] = np.where(hi, X[0], X[0] ^ t)
            X[i] = np.where(hi, X[i], X[i] ^ t)
        Q >>= np.uint64(1)
    for i in range(1, n):
        X[i] ^= X[i - 1]
    t = np.zeros(len(cell), np.uint64)
    Q = M
    while Q > np.uint64(1):
        t = np.where((X[n - 1] & Q) != 0, t ^ (Q - np.uint64(1)), t)
        Q >>= np.uint64(1)
    for i in range(n):
        X[i] ^= t
    key = np.zeros(len(cell), np.int64)
    for b in range(bits):
        for i in range(3):
            key |= np.int64(((X[i] >> np.uint64(b)) & np.uint64(1)).astype(np.int64)) << np.int64(
                3 * b + (2 - i)
            )
    return np.argsort(key, kind="stable")


def _split3(v32):
    """Exact 3-way bf16 split: v32 == hi + mid + lo (as f32 sums)."""
    import ml_dtypes

    bf = ml_dtypes.bfloat16
    hi = v32.astype(bf)
    r1 = (v32 - hi.astype(np.float32)).astype(np.float32)
    mid = r1.astype(bf)
    r2 = (r1 - mid.astype(np.float32)).astype(np.float32)
    lo = r2.astype(bf)
    recon = (
        hi.astype(np.float32) + mid.astype(np.float32) + lo.astype(np.float32)
    ).astype(np.float32)
    assert np.array_equal(recon, v32), "bf16 3-way split not exact"
    return hi.astype(np.float32), mid.astype(np.float32), lo.astype(np.float32)


def _prepare(species, coordinates, cutoff):
    """Build per-core in_maps plus host-side scatter indices."""
    import ml_dtypes

    bf = ml_dtypes.bfloat16
    coords = np.asarray(coordinates, dtype=np.float32).reshape(-1, 3).copy()
    n = coords.shape[0]
    assert n == N and n % P == 0, coords.shape
    valid = np.asarray(species).reshape(-1) >= 0
    if not valid.all():
        bad = np.where(~valid)[0]
        far = float(coords[valid].max()) if valid.any() else 0.0
        coords[bad] = (far + 20.0 + 10.0 * np.arange(len(bad), dtype=np.float32))[
            :, None
        ]

    cutf = float(cutoff)
    cut2 = np.float32(cutf) * np.float32(cutf)
    cuthi = np.nextafter(cut2, np.float32(np.inf), dtype=np.float32)
    prune2 = (cutf + 1e-3) ** 2  # conservative host-side pruning radius

    pi = _hilbert_sort(coords)
    cs = coords[pi].astype(np.float32)
    NB = n // P
    blk = np.arange(n) // P

    # candidate columns per row block, deduped at block level
    cands = []
    for R in range(NB):
        rows = cs[R * P : (R + 1) * P]
        bmin, bmax = rows.min(0), rows.max(0)
        d = np.maximum(0, np.maximum(bmin[None, :] - cs, cs - bmax[None, :]))
        cand = np.where(((d * d).sum(1) <= prune2) & (blk <= R))[0]
        cands.append(cand)

    # greedy pour, largest list first, splitting freely at piece boundaries;
    # each piece holds <= W cols and <= BANDS row-block segments
    order = sorted(range(NB), key=lambda R: -len(cands[R]))
    bins = []  # [space_left, [(R, start, width), ...]]
    cur = None
    for R in order:
        left = len(cands[R])
        s0 = 0
        while left > 0:
            if cur is None or cur[0] == 0 or len(cur[1]) == BANDS:
                bins.append([W, []])
                cur = bins[-1]
            take = min(left, cur[0])
            cur[1].append((R, s0, take))
            cur[0] -= take
            s0 += take
            left -= take
    nbins = len(bins)
    NP = max(1, -(-nbins // NCORES))

    # assign bins to cores round-robin by size
    border = sorted(range(nbins), key=lambda i: -(W - bins[i][0]))
    per_core = [[] for _ in range(NCORES)]
    for i, b in enumerate(border):
        per_core[i % NCORES].append(bins[b])

    in_maps = []
    idx_maps = []
    ccm = np.empty((P, 2), np.float32)
    ccm[:, 0] = np.float32(1e-3)
    ccm[:, 1] = cuthi
    for c in range(NCORES):
        wts_m = np.zeros((NP * K, P), np.float32)
        rhs_m = np.zeros((NP * K, W), np.float32)
        idx_m = np.full((NP, P, W), N * N, np.int64)
        for p, (_, chlist) in enumerate(per_core[c]):
            off = 0
            for band, (R, s0, w) in enumerate(chlist):
                rows = cs[R * P : (R + 1) * P]
                bmin, bmax = rows.min(0), rows.max(0)
                tR = ((bmin + bmax) * np.float32(0.5)).astype(np.float32)
                rl = (rows - tR).astype(np.float32)
                cand = cands[R][s0 : s0 + w]
                cl = (cs[cand] - tR).astype(np.float32)
                ri = ((rl[:, 0] * rl[:, 0] + rl[:, 1] * rl[:, 1]) + rl[:, 2] * rl[:, 2]).astype(np.float32)
                rj = ((cl[:, 0] * cl[:, 0] + cl[:, 1] * cl[:, 1]) + cl[:, 2] * cl[:, 2]).astype(np.float32)
                kb = p * K + band * KB
                rcol = slice(off, off + w)
                for ci in range(3):
                    rs = _split3(rl[:, ci].copy())
                    csp = _split3(cl[:, ci].copy())
                    for a in range(3):
                        wa = (np.float32(-2.0) * rs[a]).astype(bf).astype(np.float32)
                        for bb in range(3):
                            row = kb + ci * 9 + a * 3 + bb
                            wts_m[row, :] = wa
                            rhs_m[row, rcol] = csp[bb]
                for a, sp in enumerate(_split3(ri.copy())):
                    wts_m[kb + 27 + a, :] = sp
                    rhs_m[kb + 27 + a, rcol] = 1.0
                for bb, sp in enumerate(_split3(rj.copy())):
                    wts_m[kb + 30 + bb, :] = 1.0
                    rhs_m[kb + 30 + bb, rcol] = sp
                # scatter indices: orig (hi, lo) pair -> tril slot; self -> scratch
                ro = pi[R * P : (R + 1) * P]
                co = pi[cand]
                hi = np.maximum(ro[:, None], co[None, :])
                lo = np.minimum(ro[:, None], co[None, :])
                flat = hi * N + lo
                flat[ro[:, None] == co[None, :]] = N * N
                idx_m[p, :, off : off + w] = flat
                off += w
        in_maps.append(
            {
                "wts": wts_m.astype(bf),
                "rhs": rhs_m.astype(bf),
                "cc": ccm,
            }
        )
        idx_maps.append(idx_m)
    return in_maps, idx_maps, NP


def _prepare_inputs(species, coordinates, cutoff):
    in_maps, idx_maps, NP = _prepare(species, coordinates, cutoff)
    return in_maps


def _run(in_maps, trace=False):
    from concourse import bass_utils

    NP = in_maps[0]["rhs"].shape[0] // K
    nc = _get_program(NP)
    return bass_utils.run_bass_kernel_spmd(
        nc, in_maps, core_ids=list(range(NCORES)), trace=trace
    )


def _assemble(results, idx_maps):
    full = np.zeros(N * N + 1, np.float32)
    for c in range(NCORES):
        vals = results[c]["out"].astype(np.float32)
        full[idx_maps[c].ravel()] = vals.ravel()
    return full[: N * N].reshape(N, N)


def kernel(species, coordinates, cutoff):
    in_maps, idx_maps, NP = _prepare(species, coordinates, cutoff)
    res = _run(in_maps)
    return _assemble(res.results, idx_maps)


# revision 9
# speedup vs baseline: 3.4479x; 1.0672x over previous
"""Cell-list pairwise distance screen (CellList) for 8 Trainium2 NeuronCores.

Computes the masked dense [N, N] lower-triangular distance matrix:
  out[i, j] = sqrt(|c_i - c_j|^2)  if  j < i, both species valid, d2 <= cutoff^2
            = 0                    otherwise

Strategy (block-sparse + single-matmul d2):
  - Host sorts atoms along a Hilbert curve over 2.5 A cells -> 48 row blocks
    of 128 spatially-compact atoms.  For each row block R it gathers the
    candidate columns {j : dist(j, bbox_R) <= cutoff, block(j) <= R} -- a
    conservative superset of all pairs, deduplicated at block level (each
    cross-block pair appears in exactly one list; own-block pairs appear in
    both orientations of the self tile and scatter to the same output slot).
  - d2 is produced by ONE tensor-engine matmul per 512-col piece:
      d2[i,j] = ri + rj - 2*ci.cj
    expanded over exact 3-way bf16 splits of the per-block-translated
    coordinates (local coords ~ +-13 A, so f32 cancellation error ~1e-5,
    far below the ~3e-5 spacing of d2 values near cutoff^2; measured 0 mask
    flips vs the f32 reference on the target data).  33 contraction rows per
    block: 27 split cross products + 3 ri splits (x ones) + 3 rj splits.
  - Up to 3 row-block segments pack into one 512-col piece as block-diagonal
    bands (K = 99): a column's rhs rows are zero outside its own band, so
    each column only accumulates its own block's terms.  Candidate lists
    split freely across pieces/cores, so the 8 cores get an equal number of
    nearly-full pieces.
  - PSUM then holds d2 directly: one DVE bandpass (select t in (1e-3,
    cutoff^2], else 0) -> one ACT sqrt to fp16 -> DMA out.  Host scatters
    the compacted fp16 values into the full [N, N] f32 zero matrix.
  - All DRAM tensors are laid out so every per-piece DMA is one contiguous
    block (cheap descriptors): rhs [NP*K, W], wts [NP*K, P], out [NP*P, W].
"""

import threading

import numpy as np

N = 6144
P = 128
NCORES = 8
W = 512  # piece width (one PSUM bank)
KB = 33  # contraction rows per band
BANDS = 3  # bands (row-block segments) per piece
K = KB * BANDS  # 99

_lock = threading.Lock()
_cache: dict = {}


def _register_ops():
    """Register the fused DVE bandpass op at runtime (visible to table-gen)."""
    import concourse.dve_ops as dve_ops
    from concourse.dve_spec import (
        C0,
        C1,
        Spec,
        Src0,
        Zero,
        _has_src1,
        lower,
        select,
    )
    from concourse.dve_uop import DveOpSpec

    def make(name, body, ref):
        for op in dve_ops.OPS:
            if op.name == name:
                return op
        spec = Spec(body=body, reference=ref)
        row = 1 + len(dve_ops.OPS)
        assert row < 0x20
        shas = {}
        for ver in ("v3", "v4"):
            uops = lower(spec, ver=ver)
            shas[ver] = DveOpSpec(
                name=name, opcode=row, uops=uops, rd1_en=_has_src1(spec)
            ).sha(ver)
        op = dve_ops.DveOp(name, spec, subdim=False, uops_sha=shas)
        dve_ops._SUB_OPCODE_FOR_NAME[name] = row
        dve_ops.OPS.append(op)
        dve_ops.CUSTOM_DVE_SPECS[name] = spec
        return op

    # out = (s0 < in0 < s1) ? in0 : 0
    def band_ref(in0, in1, s0, s1, imm2):
        t = in0.astype(np.float32)
        keep = (t > s0) & (t < s1)
        return np.where(keep, t, 0.0).astype(np.float32)

    bandpass = make(
        "BANDPASS_ANT",
        select((Src0 > C0) & (Src0 < C1), Src0, Zero),
        band_ref,
    )
    return bandpass


def _build_program(NP, cuthi):
    import concourse.bacc as bacc
    import concourse.mybir as mybir
    import concourse.tile as tile

    bandpass = _register_ops()

    nc = bacc.Bacc("TRN2", target_bir_lowering=False, debug=False, num_devices=NCORES)
    f32 = mybir.dt.float32
    f16 = mybir.dt.float16
    bf16 = mybir.dt.bfloat16
    PW = P + W  # fused [wts | rhs] block width per piece

    NSP = (NP + 1) // 2  # 1024-col super-pieces for DVE/ACT/out-DMA
    inp = nc.dram_tensor("inp", [NP * K, PW], bf16, kind="ExternalInput")
    out = nc.dram_tensor("out", [NSP * P, 2 * W], f16, kind="ExternalOutput")

    with tile.TileContext(nc) as tc:
        with (
            tc.tile_pool(name="const", bufs=1) as cpool,
            tc.tile_pool(name="work", bufs=3) as wpool,
            tc.tile_pool(name="outp", bufs=3) as spool,
            tc.tile_pool(name="psx", bufs=3, space="PSUM") as ppx,
        ):
            cc_t = cpool.tile([P, 2], f32, tag="cc")
            inp_t = [
                cpool.tile([K, PW], bf16, tag=f"inp{p}", name=f"inp{p}")
                for p in range(NP)
            ]
            warm_t = cpool.tile([P, 2], f32, tag="warm")

            # bandpass consts via memset (no DMA); cuthi is baked per build
            nc.vector.memset(cc_t[:, 0:1], 1e-3)
            nc.gpsimd.memset(cc_t[:, 1:2], float(cuthi))

            # one contiguous DMA per piece, spread across the 3 hw issue
            # queues (vector stays free for the DVE bandpass)
            queues = [nc.sync, nc.gpsimd, nc.scalar]
            for p in range(NP):
                queues[p % 3].dma_start(inp_t[p][:], inp[p * K : (p + 1) * K, :])

            # pull the ACT sqrt table in (after the scalar-queue DMA issue)
            nc.vector.memset(warm_t[:, 0:1], 1.0)
            nc.scalar.sqrt(warm_t[:, 1:2], warm_t[:, 0:1])

            for sp in range(NSP):
                pieces = [p for p in (2 * sp, 2 * sp + 1) if p < NP]
                w = len(pieces) * W
                t = ppx.tile([P, 2 * W], f32, tag="t")
                for h, p in enumerate(pieces):
                    nc.tensor.matmul(
                        t[:, h * W : (h + 1) * W],
                        inp_t[p][:, 0:P],
                        inp_t[p][:, P:PW],
                        start=True,
                        stop=True,
                    )
                v = wpool.tile([P, 2 * W], f32, tag="v")
                nc.vector._custom_dve(
                    bandpass,
                    out=v[:, 0:w],
                    in0=t[:, 0:w],
                    s0=cc_t[:, 0:1],
                    s1=cc_t[:, 1:2],
                )
                s = spool.tile([P, 2 * W], f16, tag="s")
                nc.scalar.sqrt(s[:, 0:w], v[:, 0:w])
                q = nc.gpsimd if sp % 2 == 0 else nc.sync
                q.dma_start(out[sp * P : (sp + 1) * P, 0:w], s[:, 0:w])

    nc.compile()
    return nc


def _get_program(NP, cuthi):
    with _lock:
        key = f"nc{NP}-{float(cuthi)}"
        if key not in _cache:
            _cache[key] = _build_program(NP, cuthi)
    return _cache[key]


def _hilbert_sort(coords):
    """Atom permutation along a Hilbert curve over a 16^3 grid."""
    lo = coords.min(0)
    ext = np.maximum(coords.max(0) - lo, 1e-6)
    cell = np.clip((coords - lo) / ext * 16.0, 0, 15.999).astype(np.int64)
    X = cell.T.astype(np.uint64).copy()
    n, bits = 3, 4
    M = np.uint64(1) << np.uint64(bits - 1)
    Q = M
    while Q > np.uint64(1):
        Pm = Q - np.uint64(1)
        for i in range(n):
            hi = (X[i] & Q) != 0
            X[0] = np.where(hi, X[0] ^ Pm, X[0])
            t = (X[0] ^ X[i]) & Pm
            X[0] = np.where(hi, X[0], X[0] ^ t)
            X[i] = np.where(hi, X[i], X[i] ^ t)
        Q >>= np.uint64(1)
    for i in range(1, n):
        X[i] ^= X[i - 1]
    t = np.zeros(len(cell), np.uint64)
    Q = M
    while Q > np.uint64(1):
        t = np.where((X[n - 1] & Q) != 0, t ^ (Q - np.uint64(1)), t)
        Q >>= np.uint64(1)
    for i in range(n):
        X[i] ^= t
    key = np.zeros(len(cell), np.int64)
    for b in range(bits):
        for i in range(3):
            key |= np.int64(((X[i] >> np.uint64(b)) & np.uint64(1)).astype(np.int64)) << np.int64(
                3 * b + (2 - i)
            )
    return np.argsort(key, kind="stable")


def _split3(v32):
    """Exact 3-way bf16 split: v32 == hi + mid + lo (as f32 sums)."""
    import ml_dtypes

    bf = ml_dtypes.bfloat16
    hi = v32.astype(bf)
    r1 = (v32 - hi.astype(np.float32)).astype(np.float32)
    mid = r1.astype(bf)
    r2 = (r1 - mid.astype(np.float32)).astype(np.float32)
    lo = r2.astype(bf)
    recon = (
        hi.astype(np.float32) + mid.astype(np.float32) + lo.astype(np.float32)
    ).astype(np.float32)
    assert np.array_equal(recon, v32), "bf16 3-way split not exact"
    return hi.astype(np.float32), mid.astype(np.float32), lo.astype(np.float32)


def _prepare(species, coordinates, cutoff):
    """Build per-core in_maps plus host-side scatter indices."""
    import ml_dtypes

    bf = ml_dtypes.bfloat16
    coords = np.asarray(coordinates, dtype=np.float32).reshape(-1, 3).copy()
    n = coords.shape[0]
    assert n == N and n % P == 0, coords.shape
    valid = np.asarray(species).reshape(-1) >= 0
    if not valid.all():
        bad = np.where(~valid)[0]
        far = float(coords[valid].max()) if valid.any() else 0.0
        coords[bad] = (far + 20.0 + 10.0 * np.arange(len(bad), dtype=np.float32))[
            :, None
        ]

    cutf = float(cutoff)
    cut2 = np.float32(cutf) * np.float32(cutf)
    cuthi = np.nextafter(cut2, np.float32(np.inf), dtype=np.float32)
    prune2 = (cutf + 1e-3) ** 2  # conservative host-side pruning radius

    pi = _hilbert_sort(coords)
    cs = coords[pi].astype(np.float32)
    NB = n // P
    blk = np.arange(n) // P

    # candidate columns per row block, deduped at block level
    cands = []
    for R in range(NB):
        rows = cs[R * P : (R + 1) * P]
        bmin, bmax = rows.min(0), rows.max(0)
        d = np.maximum(0, np.maximum(bmin[None, :] - cs, cs - bmax[None, :]))
        cand = np.where(((d * d).sum(1) <= prune2) & (blk <= R))[0]
        cands.append(cand)

    # greedy pour, largest list first, splitting freely at piece boundaries;
    # each piece holds <= W cols and <= BANDS row-block segments
    order = sorted(range(NB), key=lambda R: -len(cands[R]))
    bins = []  # [space_left, [(R, start, width), ...]]
    cur = None
    for R in order:
        left = len(cands[R])
        s0 = 0
        while left > 0:
            if cur is None or cur[0] == 0 or len(cur[1]) == BANDS:
                bins.append([W, []])
                cur = bins[-1]
            take = min(left, cur[0])
            cur[1].append((R, s0, take))
            cur[0] -= take
            s0 += take
            left -= take
    nbins = len(bins)
    NP = max(1, -(-nbins // NCORES))

    # assign bins to cores round-robin by size
    border = sorted(range(nbins), key=lambda i: -(W - bins[i][0]))
    per_core = [[] for _ in range(NCORES)]
    for i, b in enumerate(border):
        per_core[i % NCORES].append(bins[b])

    in_maps = []
    idx_maps = []
    for c in range(NCORES):
        wts_m = np.zeros((NP * K, P), np.float32)
        rhs_m = np.zeros((NP * K, W), np.float32)
        idx_m = np.full(((NP + 1) // 2, P, 2 * W), N * N, np.int64)
        for p, (_, chlist) in enumerate(per_core[c]):
            off = 0
            for band, (R, s0, w) in enumerate(chlist):
                rows = cs[R * P : (R + 1) * P]
                bmin, bmax = rows.min(0), rows.max(0)
                tR = ((bmin + bmax) * np.float32(0.5)).astype(np.float32)
                rl = (rows - tR).astype(np.float32)
                cand = cands[R][s0 : s0 + w]
                cl = (cs[cand] - tR).astype(np.float32)
                ri = ((rl[:, 0] * rl[:, 0] + rl[:, 1] * rl[:, 1]) + rl[:, 2] * rl[:, 2]).astype(np.float32)
                rj = ((cl[:, 0] * cl[:, 0] + cl[:, 1] * cl[:, 1]) + cl[:, 2] * cl[:, 2]).astype(np.float32)
                kb = p * K + band * KB
                rcol = slice(off, off + w)
                for ci in range(3):
                    rs = _split3(rl[:, ci].copy())
                    csp = _split3(cl[:, ci].copy())
                    for a in range(3):
                        wa = (np.float32(-2.0) * rs[a]).astype(bf).astype(np.float32)
                        for bb in range(3):
                            row = kb + ci * 9 + a * 3 + bb
                            wts_m[row, :] = wa
                            rhs_m[row, rcol] = csp[bb]
                for a, sp in enumerate(_split3(ri.copy())):
                    wts_m[kb + 27 + a, :] = sp
                    rhs_m[kb + 27 + a, rcol] = 1.0
                for bb, sp in enumerate(_split3(rj.copy())):
                    wts_m[kb + 30 + bb, :] = 1.0
                    rhs_m[kb + 30 + bb, rcol] = sp
                # scatter indices: orig (hi, lo) pair -> tril slot; self -> scratch
                ro = pi[R * P : (R + 1) * P]
                co = pi[cand]
                hi = np.maximum(ro[:, None], co[None, :])
                lo = np.minimum(ro[:, None], co[None, :])
                flat = hi * N + lo
                flat[ro[:, None] == co[None, :]] = N * N
                cb = (p % 2) * W
                idx_m[p // 2, :, cb + off : cb + off + w] = flat
                off += w
        inp_m = np.concatenate([wts_m, rhs_m], axis=1)  # [NP*K, P+W]
        in_maps.append({"inp": inp_m.astype(bf)})
        idx_maps.append(idx_m)
    with _lock:
        _cache["cuthi"] = float(cuthi)
    return in_maps, idx_maps, NP


def _prepare_inputs(species, coordinates, cutoff):
    in_maps, idx_maps, NP = _prepare(species, coordinates, cutoff)
    return in_maps


def _run(in_maps, trace=False):
    from concourse import bass_utils

    NP = in_maps[0]["inp"].shape[0] // K
    with _lock:
        cuthi = _cache["cuthi"]
    nc = _get_program(NP, cuthi)
    return bass_utils.run_bass_kernel_spmd(
        nc, in_maps, core_ids=list(range(NCORES)), trace=trace
    )


def _assemble(results, idx_maps):
    full = np.zeros(N * N + 1, np.float32)
    for c in range(NCORES):
        vals = results[c]["out"].astype(np.float32)
        full[idx_maps[c].ravel()] = vals.ravel()
    return full[: N * N].reshape(N, N)


def kernel(species, coordinates, cutoff):
    in_maps, idx_maps, NP = _prepare(species, coordinates, cutoff)
    res = _run(in_maps)
    return _assemble(res.results, idx_maps)
